# revision 2
# baseline (speedup 1.0000x reference)
"""CPR linear on 8 TRN2 cores — lean-W + staggered first m-blocks.

Math: y = x[:, col_indices] @ (W_int8 * repeat(scales, gs)) + bias
Column permutation applied to x on the HOST; W stays natural-order so scale
rows are k-tile-aligned: device loads W as int8 (2MB) + scales [32, NS]
bf16 via partition-broadcast DMA (64KB HBM reads).

Startup exposure fix: the first m-blocks are 128/128/256/512 rows, so the
first PSUM tiles gate on ~1MB of x instead of 8MB, and W-chunk arrival
races tile 0's 6.9us k-scan instead of stalling behind a whole-slab load.

Sharding: column-parallel, 512 out cols per core; x replicated.
"""
from contextlib import ExitStack

import numpy as np
import ml_dtypes

import concourse.bass as bass
import concourse.bacc as bacc
import concourse.mybir as mybir
import concourse.tile as tile

B, S, K, N = 4, 2048, 4096, 4096
M = B * S                    # 8192
NCORES = 8
NS = N // NCORES             # 512 output cols per core
P = 128
NKT = K // P                 # 32 k-tiles
MB = 1024                    # full m-block rows
M_BLOCKS = [128, 128, 256, 512] + [1024] * 7    # sums to 8192

bf16 = mybir.dt.bfloat16
f32 = mybir.dt.float32
i8 = mybir.dt.int8

KB = 4                       # k-tiles batched per x-load DMA
NKG = NKT // KB              # 8 k-groups


def build(repeats: int = 1, variant: str = "full"):
    """variant: "full" | "whoist" | "nomm" | "mmonly"."""
    do_mm = variant != "nomm"
    do_xdma = variant != "mmonly"
    whoist = variant == "whoist"

    nc = bacc.Bacc(None)
    # x supplied pre-gathered (columns permuted) + transposed [K, M] bf16
    x_d = nc.dram_tensor("xbf", [K, M], bf16, kind="ExternalInput")
    w_d = nc.dram_tensor("wq", [K, NS], i8, kind="ExternalInput")
    s_d = nc.dram_tensor("scl", [NKT, NS], bf16, kind="ExternalInput")
    b_d = nc.dram_tensor("bias", [NS], f32, kind="ExternalInput")
    y_d = nc.dram_tensor("y", [M, NS], f32, kind="ExternalOutput")

    with tile.TileContext(nc) as tc, ExitStack() as stk:
        if repeats > 1 and not whoist:
            stk.enter_context(tc.For_i(0, repeats, 1))
        with (
            tc.tile_pool(name="consts", bufs=1) as consts,
            tc.tile_pool(name="xpool", bufs=2) as xpool,
            tc.tile_pool(name="opool", bufs=4) as opool,
            tc.tile_pool(name="psum", bufs=6, space="PSUM") as psum_pool,
        ):
            bias_t = consts.tile([P, NS], f32)
            wd = consts.tile([P, NKT * NS], bf16)
            with tc.tile_pool(name="wstage", bufs=2) as wstage:
                W_CHUNKS = [1, 1, 2, 4, 8, 8, 8]
                k0 = 0
                for h, H in enumerate(W_CHUNKS):
                    r = slice(k0 * P, (k0 + H) * P)
                    wraw = wstage.tile([P, 8, NS], i8, tag="wraw")
                    nc.scalar.dma_start(
                        out=wraw[:, :H],
                        in_=w_d[r, :].rearrange("(t p) n -> p t n", p=P))
                    sraw = wstage.tile([P, 8, NS], bf16, tag="sraw")
                    nc.scalar.dma_start(
                        out=sraw[:, :H],
                        in_=bass.AP(tensor=s_d, offset=k0 * NS,
                                    ap=[[0, P], [NS, H], [1, NS]]),
                    )
                    nc.vector.tensor_tensor(
                        out=wd[:, k0 * NS:(k0 + H) * NS],
                        in0=wraw[:, :H].opt(), in1=sraw[:, :H].opt(),
                        op=mybir.AluOpType.mult,
                    )
                    k0 += H

            nc.scalar.dma_start(
                out=bias_t,
                in_=bass.AP(tensor=b_d, offset=0, ap=[[0, P], [1, NS]]),
            )

            if repeats > 1 and whoist:
                stk.enter_context(tc.For_i(0, repeats, 1))

            xT_static = None
            if not do_xdma:
                xT_static = []
                for kg in range(NKG):
                    ts_tile = consts.tile([P, KB, MB], bf16, tag=f"xTs{kg}")
                    nc.vector.memset(ts_tile, 0.5)
                    xT_static.append(ts_tile)

            m0 = 0
            for mbw in M_BLOCKS:
                if do_xdma:
                    xT = []
                    for kg in range(NKG):
                        t = xpool.tile([P, KB, MB], bf16, tag=f"xT{kg}")
                        src = x_d[kg * KB * P:(kg + 1) * KB * P, m0:m0 + mbw]
                        nc.sync.dma_start(
                            out=t[:, :, :mbw],
                            in_=src.rearrange("(b p) m -> p b m", p=P),
                        )
                        xT.append(t)
                else:
                    xT = xT_static
                if not do_mm:
                    m0 += mbw
                    continue
                for ms in range(mbw // P):
                    ps = psum_pool.tile([P, NS], f32, tag="ps")
                    for kt in range(NKT):
                        nc.tensor.matmul(
                            ps,
                            xT[kt // KB][:, kt % KB, ms * P:(ms + 1) * P],
                            wd[:, kt * NS:(kt + 1) * NS],
                            start=(kt == 0), stop=(kt == NKT - 1),
                        )
                    ot = opool.tile([P, NS], f32, tag="ot")
                    nc.vector.tensor_tensor(
                        out=ot, in0=ps, in1=bias_t, op=mybir.AluOpType.add,
                    )
                    row0 = m0 + ms * P
                    nc.scalar.dma_start(out=y_d[row0:row0 + P, :], in_=ot)
                m0 += mbw

    nc.compile()
    return nc


def make_in_maps(x, scales, bias, weight_int8, col_indices, group_size):
    """Host-side sharding/layout prep: index gather and dtype casts only."""
    x2 = np.asarray(x, dtype=np.float32).reshape(M, K)
    ci = np.asarray(col_indices).astype(np.int64)
    x_bf = x2[:, ci].T.astype(ml_dtypes.bfloat16, order="C")   # [K, M]

    Wq = np.asarray(weight_int8).astype(np.int8)      # [K, N], exact
    sc = np.asarray(scales, dtype=np.float32)
    bias = np.asarray(bias, dtype=np.float32)

    in_maps = []
    for c in range(NCORES):
        cols = slice(c * NS, (c + 1) * NS)
        in_maps.append({
            "xbf": x_bf,
            "wq": Wq[:, cols],
            "scl": sc[:, cols].astype(ml_dtypes.bfloat16),
            "bias": bias[cols],
        })
    return in_maps


def unshard(results):
    y = np.concatenate([results[c]["y"] for c in range(NCORES)], axis=1)
    return np.ascontiguousarray(y.reshape(B, S, N))


_RUNNER = None


def _make_runner():
    """Build the bass module once and wrap it in a cached sharded jit."""
    import jax
    from jax.sharding import Mesh, PartitionSpec, NamedSharding
    from jax.experimental.shard_map import shard_map
    from concourse import bass2jax
    from concourse.bass2jax import _bass_exec_p, install_neuronx_cc_hook

    nc = build(repeats=1)
    install_neuronx_cc_hook()
    partition_name = nc.partition_id_tensor.name if nc.partition_id_tensor else None

    in_names, out_names, out_avals, zero_outs = [], [], [], []
    for alloc in nc.m.functions[0].allocations:
        if not isinstance(alloc, mybir.MemoryLocationSet):
            continue
        name = alloc.memorylocations[0].name
        if alloc.kind == "ExternalInput":
            if name != partition_name:
                in_names.append(name)
        elif alloc.kind == "ExternalOutput":
            out_names.append(name)
            shape = tuple(alloc.tensor_shape)
            dtype = mybir.dt.np(alloc.dtype)
            out_avals.append(jax.core.ShapedArray(shape, dtype))
            zero_outs.append(np.zeros(shape, dtype))
    all_in_names = list(in_names) + list(out_names)
    if partition_name is not None:
        all_in_names.append(partition_name)
    n_params, n_outs = len(in_names), len(out_names)

    def _body(*args):
        operands = list(args)
        if partition_name is not None:
            operands.append(bass2jax.partition_id_tensor())
        outs = _bass_exec_p.bind(
            *operands,
            out_avals=tuple(out_avals),
            in_names=tuple(all_in_names),
            out_names=tuple(out_names),
            lowering_input_output_aliases=(),
            sim_require_finite=True,
            sim_require_nnan=True,
            nc=nc,
        )
        return tuple(outs)

    devices = jax.devices()[:NCORES]
    mesh = Mesh(np.asarray(devices), ("core",))
    # x ("xbf") is identical on every core: pass it replicated so only one
    # copy crosses the host->device link; per-core tensors are concat-sharded.
    in_specs = tuple(
        PartitionSpec() if name == "xbf" else PartitionSpec("core")
        for name in in_names
    ) + (PartitionSpec("core"),) * n_outs
    sharded = jax.jit(
        shard_map(
            _body, mesh=mesh,
            in_specs=in_specs,
            out_specs=(PartitionSpec("core"),) * n_outs,
            check_rep=False,
        ),
        keep_unused=True,
    )
    shard_core = NamedSharding(mesh, PartitionSpec("core"))
    shard_repl = NamedSharding(mesh, PartitionSpec())

    def run(in_maps):
        import jax as _jax
        dev_in = []
        for name in in_names:
            if name == "xbf":
                dev_in.append(
                    _jax.device_put(np.asarray(in_maps[0][name]), shard_repl))
            else:
                a = np.concatenate(
                    [np.asarray(in_maps[c][name]) for c in range(NCORES)], axis=0)
                dev_in.append(_jax.device_put(a, shard_core))
        dev_zero = [
            _jax.device_put(
                np.zeros((NCORES * z.shape[0], *z.shape[1:]), z.dtype), shard_core)
            for z in zero_outs
        ]
        out = sharded(*dev_in, *dev_zero)
        return [
            {name: np.asarray(out[i]).reshape(NCORES, *zero_outs[i].shape)[c]
             for i, name in enumerate(out_names)}
            for c in range(NCORES)
        ]

    return run


def kernel(x, scales, bias, weight_int8, col_indices, group_size):
    global _RUNNER
    in_maps = make_in_maps(x, scales, bias, weight_int8, col_indices, group_size)
    if _RUNNER is None:
        _RUNNER = _make_runner()
    results = _RUNNER(in_maps)
    y = np.concatenate([results[c]["y"] for c in range(NCORES)], axis=1)
    return np.ascontiguousarray(y.reshape(B, S, N))



# revision 3
# speedup vs baseline: 1.0330x; 1.0330x over previous
"""CPR linear on 8 TRN2 cores — lean-W + staggered first m-blocks.

Math: y = x[:, col_indices] @ (W_int8 * repeat(scales, gs)) + bias
Column permutation applied to x on the HOST; W stays natural-order so scale
rows are k-tile-aligned: device loads W as int8 (2MB) + scales [32, NS]
bf16 via partition-broadcast DMA (64KB HBM reads).

Startup exposure fix: the first m-blocks are 128/128/256/512 rows, so the
first PSUM tiles gate on ~1MB of x instead of 8MB, and W-chunk arrival
races tile 0's 6.9us k-scan instead of stalling behind a whole-slab load.

Sharding: column-parallel, 512 out cols per core; x replicated.
"""
from contextlib import ExitStack

import numpy as np
import ml_dtypes

import concourse.bass as bass
import concourse.bacc as bacc
import concourse.mybir as mybir
import concourse.tile as tile

B, S, K, N = 4, 2048, 4096, 4096
M = B * S                    # 8192
NCORES = 8
NS = N // NCORES             # 512 output cols per core
P = 128
NKT = K // P                 # 32 k-tiles
MB = 1024                    # full m-block rows
M_BLOCKS = [128, 128, 256, 512] + [1024] * 7    # sums to 8192

bf16 = mybir.dt.bfloat16
f32 = mybir.dt.float32
i8 = mybir.dt.int8

KB = 4                       # k-tiles batched per x-load DMA
NKG = NKT // KB              # 8 k-groups


def build(repeats: int = 1, variant: str = "full"):
    """variant: "full" | "whoist" | "nomm" | "mmonly"."""
    do_mm = variant != "nomm"
    do_xdma = variant != "mmonly"
    whoist = variant == "whoist"

    nc = bacc.Bacc(None)
    # x supplied pre-gathered (columns permuted) + transposed [K, M] bf16
    x_d = nc.dram_tensor("xbf", [K, M], bf16, kind="ExternalInput")
    w_d = nc.dram_tensor("wq", [K, NS], i8, kind="ExternalInput")
    s_d = nc.dram_tensor("scl", [NKT, NS], bf16, kind="ExternalInput")
    b_d = nc.dram_tensor("bias", [NS], f32, kind="ExternalInput")
    y_d = nc.dram_tensor("y", [M, NS], f32, kind="ExternalOutput")

    with tile.TileContext(nc) as tc, ExitStack() as stk:
        if repeats > 1 and not whoist:
            stk.enter_context(tc.For_i(0, repeats, 1))
        with (
            tc.tile_pool(name="consts", bufs=1) as consts,
            tc.tile_pool(name="xpool", bufs=2) as xpool,
            tc.tile_pool(name="opool", bufs=4) as opool,
            tc.tile_pool(name="psum", bufs=6, space="PSUM") as psum_pool,
        ):
            bias_t = consts.tile([P, NS], f32)
            wd = consts.tile([P, NKT * NS], bf16)
            nc.scalar.dma_start(
                out=bias_t,
                in_=bass.AP(tensor=b_d, offset=0, ap=[[0, P], [1, NS]]),
            )

            with tc.tile_pool(name="wstage", bufs=2) as wstage:
                W_CHUNKS = [1, 1, 2, 4, 8, 8, 8]
                k0 = 0
                for h, H in enumerate(W_CHUNKS):
                    r = slice(k0 * P, (k0 + H) * P)
                    wraw = wstage.tile([P, 8, NS], i8, tag="wraw")
                    nc.scalar.dma_start(
                        out=wraw[:, :H],
                        in_=w_d[r, :].rearrange("(t p) n -> p t n", p=P))
                    sraw = wstage.tile([P, 8, NS], bf16, tag="sraw")
                    nc.scalar.dma_start(
                        out=sraw[:, :H],
                        in_=bass.AP(tensor=s_d, offset=k0 * NS,
                                    ap=[[0, P], [NS, H], [1, NS]]),
                    )
                    nc.vector.tensor_tensor(
                        out=wd[:, k0 * NS:(k0 + H) * NS],
                        in0=wraw[:, :H].opt(), in1=sraw[:, :H].opt(),
                        op=mybir.AluOpType.mult,
                    )
                    k0 += H

            if repeats > 1 and whoist:
                stk.enter_context(tc.For_i(0, repeats, 1))

            xT_static = None
            if not do_xdma:
                xT_static = []
                for kg in range(NKG):
                    ts_tile = consts.tile([P, KB, MB], bf16, tag=f"xTs{kg}")
                    nc.vector.memset(ts_tile, 0.5)
                    xT_static.append(ts_tile)

            m0 = 0
            for mbw in M_BLOCKS:
                if do_xdma:
                    xT = []
                    for kg in range(NKG):
                        t = xpool.tile([P, KB, MB], bf16, tag=f"xT{kg}")
                        src = x_d[kg * KB * P:(kg + 1) * KB * P, m0:m0 + mbw]
                        nc.sync.dma_start(
                            out=t[:, :, :mbw],
                            in_=src.rearrange("(b p) m -> p b m", p=P),
                        )
                        xT.append(t)
                else:
                    xT = xT_static
                if not do_mm:
                    m0 += mbw
                    continue
                for ms in range(mbw // P):
                    ps = psum_pool.tile([P, NS], f32, tag="ps")
                    for kt in range(NKT):
                        nc.tensor.matmul(
                            ps,
                            xT[kt // KB][:, kt % KB, ms * P:(ms + 1) * P],
                            wd[:, kt * NS:(kt + 1) * NS],
                            start=(kt == 0), stop=(kt == NKT - 1),
                        )
                    ot = opool.tile([P, NS], f32, tag="ot")
                    nc.vector.tensor_tensor(
                        out=ot, in0=ps, in1=bias_t, op=mybir.AluOpType.add,
                    )
                    row0 = m0 + ms * P
                    nc.scalar.dma_start(out=y_d[row0:row0 + P, :], in_=ot)
                m0 += mbw

    nc.compile()
    return nc


def make_in_maps(x, scales, bias, weight_int8, col_indices, group_size):
    """Host-side sharding/layout prep: index gather and dtype casts only."""
    x2 = np.asarray(x, dtype=np.float32).reshape(M, K)
    ci = np.asarray(col_indices).astype(np.int64)
    x_bf = x2[:, ci].T.astype(ml_dtypes.bfloat16, order="C")   # [K, M]

    Wq = np.asarray(weight_int8).astype(np.int8)      # [K, N], exact
    sc = np.asarray(scales, dtype=np.float32)
    bias = np.asarray(bias, dtype=np.float32)

    in_maps = []
    for c in range(NCORES):
        cols = slice(c * NS, (c + 1) * NS)
        in_maps.append({
            "xbf": x_bf,
            "wq": Wq[:, cols],
            "scl": sc[:, cols].astype(ml_dtypes.bfloat16),
            "bias": bias[cols],
        })
    return in_maps


def unshard(results):
    y = np.concatenate([results[c]["y"] for c in range(NCORES)], axis=1)
    return np.ascontiguousarray(y.reshape(B, S, N))


_RUNNER = None


def _make_runner():
    """Build the bass module once and wrap it in a cached sharded jit."""
    import jax
    from jax.sharding import Mesh, PartitionSpec, NamedSharding
    from jax.experimental.shard_map import shard_map
    from concourse import bass2jax
    from concourse.bass2jax import _bass_exec_p, install_neuronx_cc_hook

    nc = build(repeats=1)
    install_neuronx_cc_hook()
    partition_name = nc.partition_id_tensor.name if nc.partition_id_tensor else None

    in_names, out_names, out_avals, zero_outs = [], [], [], []
    for alloc in nc.m.functions[0].allocations:
        if not isinstance(alloc, mybir.MemoryLocationSet):
            continue
        name = alloc.memorylocations[0].name
        if alloc.kind == "ExternalInput":
            if name != partition_name:
                in_names.append(name)
        elif alloc.kind == "ExternalOutput":
            out_names.append(name)
            shape = tuple(alloc.tensor_shape)
            dtype = mybir.dt.np(alloc.dtype)
            out_avals.append(jax.core.ShapedArray(shape, dtype))
            zero_outs.append(np.zeros(shape, dtype))
    all_in_names = list(in_names) + list(out_names)
    if partition_name is not None:
        all_in_names.append(partition_name)
    n_params, n_outs = len(in_names), len(out_names)

    def _body(*args):
        operands = list(args)
        if partition_name is not None:
            operands.append(bass2jax.partition_id_tensor())
        outs = _bass_exec_p.bind(
            *operands,
            out_avals=tuple(out_avals),
            in_names=tuple(all_in_names),
            out_names=tuple(out_names),
            lowering_input_output_aliases=(),
            sim_require_finite=True,
            sim_require_nnan=True,
            nc=nc,
        )
        return tuple(outs)

    devices = jax.devices()[:NCORES]
    mesh = Mesh(np.asarray(devices), ("core",))
    # x ("xbf") is identical on every core: pass it replicated so only one
    # copy crosses the host->device link; per-core tensors are concat-sharded.
    in_specs = tuple(
        PartitionSpec() if name == "xbf" else PartitionSpec("core")
        for name in in_names
    ) + (PartitionSpec("core"),) * n_outs
    sharded = jax.jit(
        shard_map(
            _body, mesh=mesh,
            in_specs=in_specs,
            out_specs=(PartitionSpec("core"),) * n_outs,
            check_rep=False,
        ),
        keep_unused=True,
    )
    shard_core = NamedSharding(mesh, PartitionSpec("core"))
    shard_repl = NamedSharding(mesh, PartitionSpec())

    def run(in_maps):
        import jax as _jax
        dev_in = []
        for name in in_names:
            if name == "xbf":
                dev_in.append(
                    _jax.device_put(np.asarray(in_maps[0][name]), shard_repl))
            else:
                a = np.concatenate(
                    [np.asarray(in_maps[c][name]) for c in range(NCORES)], axis=0)
                dev_in.append(_jax.device_put(a, shard_core))
        dev_zero = [
            _jax.device_put(
                np.zeros((NCORES * z.shape[0], *z.shape[1:]), z.dtype), shard_core)
            for z in zero_outs
        ]
        out = sharded(*dev_in, *dev_zero)
        return [
            {name: np.asarray(out[i]).reshape(NCORES, *zero_outs[i].shape)[c]
             for i, name in enumerate(out_names)}
            for c in range(NCORES)
        ]

    return run


def kernel(x, scales, bias, weight_int8, col_indices, group_size):
    global _RUNNER
    in_maps = make_in_maps(x, scales, bias, weight_int8, col_indices, group_size)
    if _RUNNER is None:
        _RUNNER = _make_runner()
    results = _RUNNER(in_maps)
    y = np.concatenate([results[c]["y"] for c in range(NCORES)], axis=1)
    return np.ascontiguousarray(y.reshape(B, S, N))

